# revision 1
# baseline (speedup 1.0000x reference)
"""Trainium2 Bass kernel for nn_Controller (batch-1 two-layer LSTM-cell chain
+ choice head), distributed over 8 NeuronCores.

Math notes (from the module semantics): both LSTMCells run with zero initial
state, so the h @ W_hh.T terms are identically zero and the f-gate multiplies
c=0.  Only the i/g/o thirds of each W_ih are ever needed:
    gates = x @ W_ih.T + (b_ih + b_hh)
    h     = sigmoid(o) * tanh(sigmoid(i) * tanh(g))
That cuts required HBM traffic from 256 MiB to 96 MiB before sharding.

Sharding: each layer's 6144 needed gate rows are row-sharded across the 8
cores (768 rows/core, = 256 output h elements/core).  Per layer each core runs
a weights-stationary GEMV on the PE (psum output lands partition-major, which
chains straight into the next stage with no transposes).  The 1 KB h0 chunks
are AllGathered (hidden under the layer-1 weight DMA stream); the choice head
is computed as per-core partials over each core's h1 chunk, AllGathered
(8 x 128 B) and reduced on-chip.  The task mask is applied on the host.

All permutation bookkeeping from the collective layouts is folded into the
host-side weight layout prep, so the device program is just DMA + matmul +
activations.
"""

import os
import sys

import numpy as np
import ml_dtypes

for _p in ("/opt/trn_rl_repo", os.path.expanduser("~/.axon_site/_ro/trn_rl_repo")):
    if os.path.isdir(_p) and _p not in sys.path:
        sys.path.insert(0, _p)

import concourse.bass as bass
import concourse.bacc as bacc
import concourse.mybir as mybir
import concourse.tile as tile
from concourse.bass_utils import run_bass_kernel_spmd

H = 2048
NCORES = 8
C = H // NCORES          # 256: per-core h chunk
NK = H // 128            # 16 k-tiles
M6 = 6                   # 768 rows/core = 6 m-groups of 128
CH = 19                  # choice logits
AGPAD = 32               # padded per-rank chunk for the logits AllGather
DT = mybir.dt.float32
DTW = mybir.dt.bfloat16  # weight/activation-stream dtype (halves HBM traffic,
                         # single-pass PE matmul + fast weight load; adds only
                         # ~2e-4 relative error on the logits)
BF = ml_dtypes.bfloat16


# --------------------------------------------------------------------------
# host-side layout prep
# --------------------------------------------------------------------------

def _rows_k(k):
    """Global W_ih row indices (i,g,o thirds) handled by core k, in the order
    they appear along the 768-wide lhsT free axis."""
    return np.concatenate([
        0 * H + k * C + np.arange(C),
        2 * H + k * C + np.arange(C),
        3 * H + k * C + np.arange(C),
    ])


def _make_colmap():
    """x1sb[q, t] = h0[colmap[q, t]] after the AllGather + direct [128,16]
    readback. Each rank writes its [128,2] h-chunk partition-major (p*2+c),
    ranks concatenate, and the readback maps (q, t) -> flat q*16+t."""
    j = np.arange(H)
    r, rem = j // C, j % C
    perm = r * C + (rem % 2) * 128 + (rem // 2)
    return perm.reshape(128, NK)


def _host_prep(inputs):
    idx = int(np.asarray(inputs["input_idx"]).reshape(-1)[0])
    emb = np.asarray(inputs["embedding"], np.float32)
    x0 = emb[idx]
    x0T = np.ascontiguousarray(x0.reshape(NK, 128).T.astype(BF))

    colmap = _make_colmap()

    W0 = np.asarray(inputs["w_ih_0"], np.float32)
    W1 = np.asarray(inputs["w_ih_1"], np.float32)
    B0 = np.asarray(inputs["b_ih_0"], np.float32) + np.asarray(inputs["b_hh_0"], np.float32)
    B1 = np.asarray(inputs["b_ih_1"], np.float32) + np.asarray(inputs["b_hh_1"], np.float32)
    WC = np.asarray(inputs["w_choice"], np.float32)
    BC = np.asarray(inputs["b_choice"], np.float32)

    maps = []
    for k in range(NCORES):
        R = _rows_k(k)
        w0h = np.ascontiguousarray(W0[R].T.reshape(NK, 128, 3 * C).astype(BF))
        b0h = np.ascontiguousarray(B0[R].reshape(M6, 128).T)
        w1h = np.ascontiguousarray(np.transpose(W1[R][:, colmap], (2, 1, 0)).astype(BF))
        b1h = np.ascontiguousarray(B1[R].reshape(M6, 128).T)
        wcs = WC[:, k * C:(k + 1) * C].reshape(CH, 2, 128)
        wch = np.ascontiguousarray(
            np.transpose(wcs, (2, 1, 0)).reshape(128, 2 * CH).astype(BF))
        bch = np.ascontiguousarray(BC.reshape(1, CH))
        maps.append(dict(x0T=x0T, w0=w0h, b0=b0h, w1=w1h, b1=b1h, wc=wch, bc=bch))
    return maps


# --------------------------------------------------------------------------
# device program (identical on all 8 cores; per-core data differs)
# --------------------------------------------------------------------------

def _gemv_layer(nc, wp, pp, ap, w_dram, wtag, x_sb, b_sb):
    """768-row weights-stationary GEMV + bias + LSTM-cell activations.
    Returns h tile [128, 2] (partition-major h-chunk)."""
    psums = [pp.tile([128, 1], DT, tag=f"ps{m}", name=f"{wtag}_ps{m}")
             for m in range(M6)]
    wtiles = []
    for t in range(NK):
        wt = wp.tile([128, 3 * C], DTW, tag=f"{wtag}_{t}", name=f"{wtag}_t{t}")
        nc.sync.dma_start(wt[:], w_dram[t])
        wtiles.append(wt)
    for t in range(NK):
        for m in range(M6):
            nc.tensor.matmul(
                psums[m][:],
                wtiles[t][:, m * 128:(m + 1) * 128],
                x_sb[:, t:t + 1],
                start=(t == 0),
                stop=(t == NK - 1),
            )
    g = ap.tile([128, M6], DT, tag=f"{wtag}_g", name=f"{wtag}_g")
    for m in range(M6):
        nc.vector.tensor_add(g[:, m:m + 1], psums[m][:], b_sb[:, m:m + 1])
    sig_i = ap.tile([128, 2], DT, name=f"{wtag}_sig_i", tag=f"{wtag}_si")
    tanh_g = ap.tile([128, 2], DT, name=f"{wtag}_tanh_g", tag=f"{wtag}_tg")
    cst = ap.tile([128, 2], DT, name=f"{wtag}_cst", tag=f"{wtag}_c")
    tanh_c = ap.tile([128, 2], DT, name=f"{wtag}_tanh_c", tag=f"{wtag}_tc")
    sig_o = ap.tile([128, 2], DT, name=f"{wtag}_sig_o", tag=f"{wtag}_so")
    h = ap.tile([128, 2], DTW, name=f"{wtag}_h", tag=f"{wtag}_h")
    Act = mybir.ActivationFunctionType
    nc.scalar.activation(sig_i[:], g[:, 0:2], Act.Sigmoid)
    nc.scalar.activation(tanh_g[:], g[:, 2:4], Act.Tanh)
    nc.vector.tensor_mul(cst[:], sig_i[:], tanh_g[:])
    nc.scalar.activation(tanh_c[:], cst[:], Act.Tanh)
    nc.scalar.activation(sig_o[:], g[:, 4:6], Act.Sigmoid)
    nc.vector.tensor_mul(h[:], tanh_c[:], sig_o[:])
    return h


def _build_nc():
    nc = bacc.Bacc("TRN2", target_bir_lowering=False, debug=False,
                   num_devices=NCORES)

    x0T = nc.dram_tensor("x0T", [128, NK], DTW, kind="ExternalInput")
    w0 = nc.dram_tensor("w0", [NK, 128, 3 * C], DTW, kind="ExternalInput")
    b0 = nc.dram_tensor("b0", [128, M6], DT, kind="ExternalInput")
    w1 = nc.dram_tensor("w1", [NK, 128, 3 * C], DTW, kind="ExternalInput")
    b1 = nc.dram_tensor("b1", [128, M6], DT, kind="ExternalInput")
    wc = nc.dram_tensor("wc", [128, 2 * CH], DTW, kind="ExternalInput")
    bc = nc.dram_tensor("bc", [1, CH], DT, kind="ExternalInput")
    out = nc.dram_tensor("out", [CH], DT, kind="ExternalOutput")

    rg = [list(range(NCORES))]

    with tile.TileContext(nc) as tc:
        with (
            tc.tile_pool(name="weights", bufs=1) as wp,
            tc.tile_pool(name="small", bufs=1) as sp,
            tc.tile_pool(name="act", bufs=1) as ap,
            tc.tile_pool(name="psum", bufs=1, space=bass.MemorySpace.PSUM) as pp,
            tc.tile_pool(name="dram", bufs=1, space=bass.MemorySpace.DRAM) as dp,
        ):
            # small loads go through gpsimd (SWDGE) so the sync-engine FIFO
            # stays a pure, never-stalling weight stream
            x0sb = sp.tile([128, NK], DTW, tag="x0")
            nc.gpsimd.dma_start(x0sb[:], x0T[:])
            b0sb = sp.tile([128, M6], DT, tag="b0")
            nc.gpsimd.dma_start(b0sb[:], b0[:])
            b1sb = sp.tile([128, M6], DT, tag="b1")
            nc.gpsimd.dma_start(b1sb[:], b1[:])
            wcsb = sp.tile([128, 2 * CH], DTW, tag="wc")
            nc.gpsimd.dma_start(wcsb[:], wc[:])
            bcsb = sp.tile([1, CH], DT, tag="bc")
            nc.gpsimd.dma_start(bcsb[:], bc[:])

            # ---- layer 0 ----
            h0 = _gemv_layer(nc, wp, pp, ap, w0, "w0", x0sb, b0sb)

            # ---- AllGather h0 chunks ----
            cc1_in = dp.tile([C], DTW, tag="cc1_in")
            cc1_out = dp.tile([H], DTW, tag="cc1_out")
            nc.gpsimd.dma_start(cc1_in.rearrange("(p c) -> p c", c=2), h0[:])
            nc.gpsimd.collective_compute(
                "AllGather", mybir.AluOpType.bypass,
                ins=[cc1_in.opt()], outs=[cc1_out.opt()], replica_groups=rg,
            )
            x1sb = sp.tile([128, NK], DTW, tag="x1")
            nc.gpsimd.dma_start(x1sb[:], cc1_out.rearrange("(q t) -> q t", t=NK))

            # ---- layer 1 ----
            h1 = _gemv_layer(nc, wp, pp, ap, w1, "w1", x1sb, b1sb)

            # ---- choice-head partials over this core's h1 chunk ----
            ps_head = pp.tile([CH, 1], DT, tag="head")
            for c in range(2):
                nc.tensor.matmul(
                    ps_head[:], wcsb[:, c * CH:(c + 1) * CH], h1[:, c:c + 1],
                    start=(c == 0), stop=(c == 1),
                )
            padded = ap.tile([AGPAD, 1], DT, tag="headpad")
            nc.gpsimd.memset(padded[:], 0.0)
            nc.vector.tensor_copy(padded[0:CH, :], ps_head[:])

            cc2_in = dp.tile([AGPAD], DT, tag="cc2_in")
            cc2_out = dp.tile([AGPAD * NCORES], DT, tag="cc2_out")
            nc.gpsimd.dma_start(cc2_in.rearrange("(p c) -> p c", c=1), padded[:])
            nc.gpsimd.collective_compute(
                "AllGather", mybir.AluOpType.bypass,
                ins=[cc2_in.opt()], outs=[cc2_out.opt()], replica_groups=rg,
            )

            # ---- reduce the 8 partials + bias, write logits ----
            parts = sp.tile([1, AGPAD * NCORES], DT, tag="parts")
            nc.gpsimd.dma_start(parts[:], cc2_out.rearrange("(a n) -> a n", a=1))
            acc = ap.tile([1, CH], DT, tag="acc")
            nc.vector.tensor_add(acc[:], parts[:, 0:CH], bcsb[:])
            for r in range(1, NCORES):
                nc.vector.tensor_add(acc[:], acc[:], parts[:, r * AGPAD:r * AGPAD + CH])
            nc.gpsimd.dma_start(out.rearrange("(a n) -> a n", a=1), acc[:])

    nc.compile()
    return nc


_NC_CACHE = None


def _get_nc():
    global _NC_CACHE
    if _NC_CACHE is None:
        _NC_CACHE = _build_nc()
    return _NC_CACHE


# --------------------------------------------------------------------------
# entry point
# --------------------------------------------------------------------------

def kernel(**inputs) -> np.ndarray:
    task = int(np.asarray(inputs["task"]).reshape(-1)[0]) if not isinstance(
        inputs["task"], int) else int(inputs["task"])
    maps = _host_prep(inputs)
    nc = _get_nc()
    for attempt in range(3):
        res = run_bass_kernel_spmd(nc, maps, list(range(NCORES)))
        outs = [np.asarray(res.results[i]["out"], np.float32).reshape(CH)
                for i in range(NCORES)]
        # post-AllGather every core holds identical logits; disagreement means
        # the device was in a bad state -- retry
        if all(np.array_equal(outs[0], o) for o in outs[1:]):
            break
    logits = outs[0]
    mask = np.arange(CH) < (1 + task)
    return np.where(mask, logits, np.float32(-1e9)).astype(np.float32)


if __name__ == "__main__":
    import reference  # only for standalone debugging; not used by the grader

    inputs = reference.setup_inputs()
    expected = np.asarray(reference.reference(**inputs))
    actual = kernel(**inputs)
    print("expected:", expected)
    print("actual:  ", actual)
    denom = np.abs(expected).max()
    print("max abs err:", np.abs(actual - expected).max(),
          "rel:", np.abs(actual - expected).max() / denom)



# revision 3
# speedup vs baseline: 1.2750x; 1.2750x over previous
"""Trainium2 Bass kernel for nn_Controller (batch-1 two-layer LSTM-cell chain
+ choice head), distributed over 8 NeuronCores.

Math notes: both LSTMCells run with zero initial state, so h @ W_hh.T == 0 and
the f-gate multiplies c=0.  Only the i/g/o thirds of each W_ih matter:
    gates = x @ W_ih.T + (b_ih + b_hh)
    h     = sigmoid(o) * tanh(sigmoid(i) * tanh(g))

Structure (one collective total):
  - Layer 0 row-sharded: core k computes gate rows for its h0 chunk (256 h
    elems = 768 i/g/o rows) as a flipped GEMV: the x column is the PE's
    stationary operand (1-column weight loads) and the weight tile streams as
    the moving operand, 512/256-wide.  Bias is folded into the accumulation
    group via a K=1 matmul against a ones scalar.
  - h0 chunk [1,256] is transposed to contraction layout [128,2] with two K=1
    matmuls against ones (out[128,1] = h0[1,128].T @ [1]).
  - Layer 1 contraction-sharded: core k multiplies its h0 chunk into
    W_ih_1[:, chunk] producing partial pre-activations for ALL 6144 gates.
    No inter-layer collective needed.
  - One AllGather of the [6144] f32 partials; each core tree-reduces the 8
    copies on DVE, adds the (once-only) bias, applies the LSTM activations
    across 128 partitions, and computes the 19-logit head locally.
All permutation bookkeeping is folded into host-side weight layout prep.
"""

import os
import sys

import numpy as np
import ml_dtypes

for _p in ("/opt/trn_rl_repo", os.path.expanduser("~/.axon_site/_ro/trn_rl_repo")):
    if os.path.isdir(_p) and _p not in sys.path:
        sys.path.insert(0, _p)

import concourse.bass as bass
import concourse.bacc as bacc
import concourse.mybir as mybir
import concourse.tile as tile
from concourse.bass_utils import run_bass_kernel_spmd

H = 2048
NCORES = 8
C = H // NCORES          # 256: per-core h chunk
NK = H // 128            # 16 k-tiles
CH = 19                  # choice logits
G1 = 3 * H               # 6144 layer-1 gates (i,g,o)
DT = mybir.dt.float32
DTW = mybir.dt.bfloat16
BF = ml_dtypes.bfloat16

W0_COLS = 16 + NK * 768          # x0T (16) + 16 k-tiles x 768 gate cols
W0_CHUNK = [(0, 3088), (3088, 6160), (6160, 9232), (9232, 12304)]
W1_COLS = 12 * 1024              # 12 n-chunks x (2 k-subs x 512)
W1_CHUNK = [(0, 3072), (3072, 6144), (6144, 9216), (9216, 12288)]


# --------------------------------------------------------------------------
# host-side layout prep
# --------------------------------------------------------------------------

def _gate_rows_all():
    """W_ih row indices for the i,g,o gates of h elems 0..H-1, grouped
    per-core: rows [k*768:(k+1)*768] = [i(256) | g(256) | o(256)] of core k."""
    out = []
    for k in range(NCORES):
        e = k * C + np.arange(C)
        out.append(np.concatenate([e, 2 * H + e, 3 * H + e]))
    return np.concatenate(out)


def _l1_perm():
    """Gate order f for the AllGather buffer: f = p*48 + a*16 + t holds gate
    a (0=i,1=g,2=o) of h1 element e = t*128 + p.  Readback as [128,48] then
    puts gates (i|g|o) x 16 k-cols on each partition."""
    f = np.arange(G1)
    p, rem = f // 48, f % 48
    a, t = rem // 16, rem % 16
    e = t * 128 + p
    return np.array([0, 2 * H, 3 * H])[a] + e  # W_ih_1 row for each f


def _host_prep(inputs):
    idx = int(np.asarray(inputs["input_idx"]).reshape(-1)[0])
    emb = np.asarray(inputs["embedding"], np.float32)
    x0 = emb[idx]
    x0T = np.ascontiguousarray(x0.reshape(NK, 128).T.astype(BF))  # [128,16]

    W0 = np.asarray(inputs["w_ih_0"], np.float32)
    W1 = np.asarray(inputs["w_ih_1"], np.float32)
    B0 = np.asarray(inputs["b_ih_0"], np.float32) + np.asarray(inputs["b_hh_0"], np.float32)
    B1 = np.asarray(inputs["b_ih_1"], np.float32) + np.asarray(inputs["b_hh_1"], np.float32)
    WC = np.asarray(inputs["w_choice"], np.float32)
    BC = np.asarray(inputs["b_choice"], np.float32)

    rows0 = _gate_rows_all()
    W0g = W0[rows0]                       # [8*768, 2048]
    B0g = B0[rows0]                       # [8*768]

    fperm = _l1_perm()
    W1g = W1[fperm].astype(BF)            # [6144, 2048] bf16
    biasg = np.ascontiguousarray(B1[fperm].reshape(128, 48))  # [128,48] f32

    # wc[p, t*19+j] = w_choice[j, t*128+p]
    wch = np.ascontiguousarray(
        np.transpose(WC.reshape(CH, NK, 128), (2, 1, 0)).reshape(128, NK * CH)
        .astype(BF))

    maps = []
    for k in range(NCORES):
        Wk = W0g[k * 768:(k + 1) * 768]                      # [768, 2048]
        blk = Wk.T.reshape(NK, 128, 768)                     # [t, p, n]
        w0flat = np.transpose(blk, (1, 0, 2)).reshape(128, NK * 768).astype(BF)
        w0h = np.ascontiguousarray(np.concatenate([x0T, w0flat], axis=1))

        b0h = np.ascontiguousarray(
            np.concatenate([B0g[k * 768:(k + 1) * 768], BC]).reshape(1, 787))

        # w1flat[q, n*1024 + c*512 + j] = W1g[n*512+j, k*256 + c*128 + q]
        sel = W1g[:, k * C:(k + 1) * C]                      # [6144, 256] bf16
        arr = sel.reshape(12, 512, 2, 128)                   # [n, j, c, q]
        w1h = np.ascontiguousarray(
            np.transpose(arr, (3, 0, 2, 1)).reshape(128, W1_COLS))

        maps.append(dict(w0=w0h, w1=w1h, p0c=b0h, biasg=biasg, wc=wch))
    return maps


# --------------------------------------------------------------------------
# device program (identical on all 8 cores; per-core data differs)
# --------------------------------------------------------------------------

def _build_nc():
    nc = bacc.Bacc("TRN2", target_bir_lowering=False, debug=False,
                   num_devices=NCORES)

    w0d = nc.dram_tensor("w0", [128, W0_COLS], DTW, kind="ExternalInput")
    w1d = nc.dram_tensor("w1", [128, W1_COLS], DTW, kind="ExternalInput")
    p0cd = nc.dram_tensor("p0c", [1, 787], DT, kind="ExternalInput")
    biasgd = nc.dram_tensor("biasg", [128, 48], DT, kind="ExternalInput")
    wcd = nc.dram_tensor("wc", [128, NK * CH], DTW, kind="ExternalInput")
    out = nc.dram_tensor("out", [CH], DT, kind="ExternalOutput")

    rg = [list(range(NCORES))]
    Act = mybir.ActivationFunctionType

    with tile.TileContext(nc) as tc:
        with (
            tc.tile_pool(name="weights", bufs=1) as wp,
            tc.tile_pool(name="small", bufs=1) as sp,
            tc.tile_pool(name="act", bufs=1) as ap,
            tc.tile_pool(name="psum", bufs=1, space=bass.MemorySpace.PSUM) as pp,
            tc.tile_pool(name="dram", bufs=1, space=bass.MemorySpace.DRAM) as dp,
        ):
            # ---- weight streams (HWDGE, big chunks) ----
            wt = []
            for c, (a, b) in enumerate(W0_CHUNK):
                t_ = wp.tile([128, b - a], DTW, tag=f"w0c{c}", name=f"w0c{c}")
                nc.sync.dma_start(t_[:], w0d[:, a:b])
                wt.append(t_)
            vt = []
            for c, (a, b) in enumerate(W1_CHUNK):
                t_ = wp.tile([128, b - a], DTW, tag=f"w1c{c}", name=f"w1c{c}")
                nc.sync.dma_start(t_[:], w1d[:, a:b])
                vt.append(t_)

            # ---- small loads (SWDGE) + on-chip constants ----
            p0c = sp.tile([1, 787], DT, tag="p0c")
            nc.gpsimd.dma_start(p0c[:], p0cd[:])
            biasg = sp.tile([128, 48], DT, tag="biasg")
            nc.gpsimd.dma_start(biasg[:], biasgd[:])
            wcsb = sp.tile([128, NK * CH], DTW, tag="wc")
            nc.gpsimd.dma_start(wcsb[:], wcd[:])
            ones32 = sp.tile([1, 1], DT, tag="ones32")
            nc.gpsimd.memset(ones32[:], 1.0)
            ones16 = sp.tile([1, 1], DTW, tag="ones16")
            nc.gpsimd.memset(ones16[:], 1.0)

            def w0ap(t, n0, n1):
                """AP for k-tile t, gate cols [n0:n1) of layer-0 weights."""
                c = t // 4
                base = (16 if c == 0 else 0) + (t % 4) * 768
                return wt[c][:, base + n0: base + n1]

            def x0col(t):
                return wt[0][:, t:t + 1]

            # ---- layer 0: gates [1,768] = x0 @ W0ihT + b0, flipped GEMV ----
            psA = pp.tile([1, 512], DT, tag="psA")
            psB = pp.tile([1, 256], DT, tag="psB")
            for t in range(NK):
                nc.tensor.matmul(psA[:], x0col(t), w0ap(t, 0, 512),
                                 start=(t == 0), stop=False)
                nc.tensor.matmul(psB[:], x0col(t), w0ap(t, 512, 768),
                                 start=(t == 0), stop=False)
            nc.tensor.matmul(psA[:], ones32[:], p0c[0:1, 0:512],
                             start=False, stop=True)
            nc.tensor.matmul(psB[:], ones32[:], p0c[0:1, 512:768],
                             start=False, stop=True)

            # ---- layer-0 LSTM activations on [1,256] slices ----
            sig_i = ap.tile([1, 256], DT, tag="sig_i")
            nc.scalar.activation(sig_i[:], psA[0:1, 0:256], Act.Sigmoid)
            tanh_g = ap.tile([1, 256], DT, tag="tanh_g")
            nc.scalar.activation(tanh_g[:], psA[0:1, 256:512], Act.Tanh)
            sig_o = ap.tile([1, 256], DT, tag="sig_o")
            nc.scalar.activation(sig_o[:], psB[0:1, 0:256], Act.Sigmoid)
            cst = ap.tile([1, 256], DT, tag="cst")
            nc.vector.tensor_mul(cst[:], sig_i[:], tanh_g[:])
            tanh_c = ap.tile([1, 256], DT, tag="tanh_c")
            nc.scalar.activation(tanh_c[:], cst[:], Act.Tanh)
            h0 = ap.tile([1, 256], DTW, tag="h0")
            nc.vector.tensor_mul(h0[:], tanh_c[:], sig_o[:])

            # ---- transpose h0 [1,256] -> x1 [128,2] via K=1 matmuls ----
            psT = pp.tile([128, 2], DT, tag="psT")
            for c in range(2):
                nc.tensor.matmul(psT[:, c:c + 1], h0[0:1, c * 128:(c + 1) * 128],
                                 ones16[:], start=True, stop=True)
            x1 = ap.tile([128, 2], DTW, tag="x1")
            nc.vector.tensor_copy(x1[:], psT[:])

            # ---- layer 1: partial gates [1,6144] over this core's h0 chunk --
            psL = [pp.tile([1, 512], DT, tag=f"psL{b}", name=f"psL{b}")
                   for b in range(4)]
            partials = sp.tile([1, G1], DT, tag="partials")
            for n in range(12):
                b = n % 4
                ch = vt[n // 3]
                base = (n % 3) * 1024
                for c in range(2):
                    nc.tensor.matmul(
                        psL[b][:], x1[:, c:c + 1],
                        ch[:, base + c * 512: base + (c + 1) * 512],
                        start=(c == 0), stop=(c == 1))
                dst = partials[0:1, n * 512:(n + 1) * 512]
                if n % 2 == 0:
                    nc.vector.tensor_copy(dst, psL[b][:])
                else:
                    nc.scalar.activation(dst, psL[b][:], Act.Copy)

            # ---- single AllGather of the f32 partials ----
            cc_in = dp.tile([G1], DT, tag="cc_in")
            nc.scalar.dma_start(cc_in.rearrange("(a n) -> a n", a=1), partials[:])
            cc_out = dp.tile([NCORES * G1], DT, tag="cc_out")
            nc.gpsimd.collective_compute(
                "AllGather", mybir.AluOpType.bypass,
                ins=[cc_in.opt()], outs=[cc_out.opt()], replica_groups=rg,
            )

            # ---- readback [128, 8, 48] + tree reduce over ranks + bias ----
            R = sp.tile([128, NCORES, 48], DT, tag="R")
            nc.scalar.dma_start(
                R[:], cc_out.rearrange("(r p j) -> p r j", r=NCORES, p=128))
            S1 = ap.tile([128, 4, 48], DT, tag="S1")
            nc.vector.tensor_add(S1[:], R[:, 0:4, :], R[:, 4:8, :])
            S2 = ap.tile([128, 2, 48], DT, tag="S2")
            nc.vector.tensor_add(S2[:], S1[:, 0:2, :], S1[:, 2:4, :])
            S3 = ap.tile([128, 48], DT, tag="S3")
            nc.vector.tensor_add(S3[:], S2[:, 0, :], S2[:, 1, :])
            G = ap.tile([128, 48], DT, tag="G")
            nc.vector.tensor_add(G[:], S3[:], biasg[:])

            # ---- layer-1 LSTM activations across 128 partitions ----
            sig_i1 = ap.tile([128, 16], DT, tag="sig_i1")
            nc.scalar.activation(sig_i1[:], G[:, 0:16], Act.Sigmoid)
            tanh_g1 = ap.tile([128, 16], DT, tag="tanh_g1")
            nc.scalar.activation(tanh_g1[:], G[:, 16:32], Act.Tanh)
            sig_o1 = ap.tile([128, 16], DT, tag="sig_o1")
            nc.scalar.activation(sig_o1[:], G[:, 32:48], Act.Sigmoid)
            cst1 = ap.tile([128, 16], DT, tag="cst1")
            nc.vector.tensor_mul(cst1[:], sig_i1[:], tanh_g1[:])
            tanh_c1 = ap.tile([128, 16], DT, tag="tanh_c1")
            nc.scalar.activation(tanh_c1[:], cst1[:], Act.Tanh)
            h1 = ap.tile([128, 16], DTW, tag="h1")
            nc.vector.tensor_mul(h1[:], tanh_c1[:], sig_o1[:])

            # ---- choice head: logits [1,19] = h1 . Wc + bc ----
            psH = pp.tile([1, CH], DT, tag="psH")
            for t in range(NK):
                nc.tensor.matmul(psH[:], h1[:, t:t + 1],
                                 wcsb[:, t * CH:(t + 1) * CH],
                                 start=(t == 0), stop=False)
            nc.tensor.matmul(psH[:], ones32[:], p0c[0:1, 768:787],
                             start=False, stop=True)
            logit = ap.tile([1, CH], DT, tag="logit")
            nc.vector.tensor_copy(logit[:], psH[:])
            nc.scalar.dma_start(out.rearrange("(a n) -> a n", a=1), logit[:])

    nc.compile()
    return nc


_NC_CACHE = None


def _get_nc():
    global _NC_CACHE
    if _NC_CACHE is None:
        _NC_CACHE = _build_nc()
    return _NC_CACHE


# --------------------------------------------------------------------------
# entry point
# --------------------------------------------------------------------------

def kernel(**inputs) -> np.ndarray:
    task = int(np.asarray(inputs["task"]).reshape(-1)[0]) if not isinstance(
        inputs["task"], int) else int(inputs["task"])
    maps = _host_prep(inputs)
    nc = _get_nc()
    for attempt in range(3):
        res = run_bass_kernel_spmd(nc, maps, list(range(NCORES)))
        outs = [np.asarray(res.results[i]["out"], np.float32).reshape(CH)
                for i in range(NCORES)]
        # post-AllGather every core holds identical logits; disagreement means
        # the device was in a bad state -- retry
        if all(np.array_equal(outs[0], o) for o in outs[1:]):
            break
    logits = outs[0]
    mask = np.arange(CH) < (1 + task)
    return np.where(mask, logits, np.float32(-1e9)).astype(np.float32)


if __name__ == "__main__":
    import reference  # only for standalone debugging; not used by the grader

    inputs = reference.setup_inputs()
    expected = np.asarray(reference.reference(**inputs))
    actual = kernel(**inputs)
    print("expected:", expected)
    print("actual:  ", actual)
    denom = np.abs(expected).max()
    print("max abs err:", np.abs(actual - expected).max(),
          "rel:", np.abs(actual - expected).max() / denom)
